# revision 9
# baseline (speedup 1.0000x reference)
"""Trainium2 Bass kernel for GQA sliding-window causal attention.

Problem: B=2, S=2048, H=32 q-heads, KVH=8 kv-heads, D=128,
sliding window 1024, causal, scale 1/sqrt(128). f32 I/O.

Sharding (8 cores, pure tensor parallel, no collectives): core c gets
kv-head c and its query-head group [4c, 4c+4). Each core computes full
attention for its 4 q-heads over both batch elements; host concatenates
along the head dim.

Per-core algorithm (banded, no online softmax needed since scores are
O(1) and exp never overflows):
  - Q and K live in SBUF transposed: [d=128 partitions, s free].
  - Scores computed transposed, ST[k, q] = (KT_j).T-contracted-with-QT,
    per (512-wide q-block, 128-wide k-tile) over the causal+window band.
    Two consecutive k-tiles share one 2-bank PSUM mega-tile so a single
    ScalarE activation handles both (amortizes the ~300-cycle ACT
    instruction overhead).
  - P = exp(SCALE * ST - 2) on ScalarE, written as fp8e4 to SBUF. The
    -2 offset keeps exp below the fp8e4 max (240); it cancels in the
    final normalization.
  - Causal-diagonal and window-edge tiles are masked AFTER exp by
    multiplying with 0/1 fp8 mask tiles on VectorE (exact zeros).
  - PV: acc[q, 0:129] += PT_slice.T @ V'_j where PT is fp8 (stationary,
    4x faster weight load) and V' is bf16 with a ones column appended
    -> col 128 accumulates the softmax denominator for free.
    Two q-tiles' accumulators share one PSUM bank (2*129 <= 512); the
    chronologically first matmul into a bank carries start=True (whole-
    bank has_written clear), everything after accumulates per element.
  - acc banks are copied raw (numerator + denominator) to SBUF on DVE
    and DMA'd out; the division happens on the host during unsharding.
All matmuls accumulate f32 in PSUM; softmax math in f32 on ScalarE.
"""

import numpy as np
import ml_dtypes

B = 2
S = 2048
H = 32
KVH = 8
D = 128
HQ = H // KVH  # q heads per core = 4
W = 1024  # sliding window
SCALE = 0.08838834764831845
EXP_BIAS = -3.5  # folded into exp; cancels in normalization.
# Keeps exp below the fp8e4 max (240): observed max scaled score is ~7.8
# (driven by aligned large-norm q/k rows), overflow would need > 8.98.
N_CORES = 8
BS = B * S  # 4096
NT = S // 128  # 16 k-tiles / q-tiles per sequence
NG = S // 512  # 4 q-blocks per sequence
VW = D + 1  # 129: V width with ones column
OW = 2 * VW  # 258: two packed (num|den) column groups per PSUM bank

_BF16 = ml_dtypes.bfloat16
_FP8 = ml_dtypes.float8_e4m3

_CACHE = {}


def _pairs_for_g(g):
    """Consecutive-j pairs for q-block g, larger-n first within a pair.

    Returns list of [(j, n, qv, qe), (j, n, qv, qe)] pairs covering the
    causal+window band for q-range [512g, 512g+512).
    """
    q0 = 512 * g
    tiles = []
    for j in range(max(0, 4 * g - 8), 4 * g + 4):
        qv = max(q0, 128 * j)
        qe = min(q0 + 512, 128 * j + 128 + W)
        tiles.append((j, qe - qv, qv, qe))
    pairs = []
    for t in range(0, len(tiles), 2):
        a, b = tiles[t], tiles[t + 1]
        if a[1] < b[1]:
            a, b = b, a  # larger n first: avoids bank-crossing placement
        pairs.append((a, b))
    return pairs


def _build_nc(reps=1, loop_reps=0, opts=None):
    """Build + compile the single-core Bass/Tile program (SPMD across 8).

    reps > 1 unrolls the whole computation inside one NEFF; loop_reps > 0
    instead wraps the body in a hardware For_i loop. Both are used only
    for timing. opts: dict of tuning switches (see _body_once).
    """
    from contextlib import ExitStack

    import concourse.bass as bass
    import concourse.tile as tile
    from concourse import bacc, mybir

    opts = dict(opts or {})
    fp32 = mybir.dt.float32
    bf16 = mybir.dt.bfloat16
    fp8 = mybir.dt.float8e4
    p_dt = fp8 if opts.get("fp8_p", False) else bf16

    nc = bacc.Bacc("TRN2", target_bir_lowering=False, debug=False,
                   num_devices=N_CORES)

    qt_d = nc.dram_tensor("qt", [HQ, D, BS], bf16, kind="ExternalInput").ap()
    kt_d = nc.dram_tensor("kt", [D, BS], bf16, kind="ExternalInput").ap()
    vv_d = nc.dram_tensor("vv", [B, 128, NT * VW], bf16, kind="ExternalInput").ap()
    mk_d = nc.dram_tensor("mk", [128, 256], p_dt, kind="ExternalInput").ap()
    out_d = nc.dram_tensor("out", [HQ, B, NG, 2, 128, OW], fp32,
                           kind="ExternalOutput").ap()

    with tile.TileContext(nc) as tc, ExitStack() as ctx:
        mask_pool = ctx.enter_context(tc.tile_pool(name="mask", bufs=1))
        kt_pool = ctx.enter_context(tc.tile_pool(name="ktp", bufs=2))
        vv_pool = ctx.enter_context(tc.tile_pool(name="vvp", bufs=2))
        qt_pool = ctx.enter_context(tc.tile_pool(name="qtp", bufs=2))
        pt_pool = ctx.enter_context(
            tc.tile_pool(name="ptp", bufs=opts.get("pt_bufs", 3)))
        osb_pool = ctx.enter_context(tc.tile_pool(name="osb", bufs=4))
        st_pool = ctx.enter_context(
            tc.tile_pool(name="stp", bufs=opts.get("st_bufs", 3), space="PSUM"))
        acc_pool = ctx.enter_context(
            tc.tile_pool(name="accp", bufs=opts.get("acc_bufs", 1),
                         space="PSUM"))

        masks = mask_pool.tile([128, 256], p_dt)
        nc.sync.dma_start(masks[:], mk_d[:])
        ebias = mask_pool.tile([128, 1], fp32, name="ebias")
        nc.vector.memset(ebias[:], EXP_BIAS)

        pools = (kt_pool, vv_pool, qt_pool, pt_pool, osb_pool, st_pool,
                 acc_pool)
        if loop_reps:
            with tc.For_i(0, loop_reps, 1,
                          hint_engines=tuple(nc.engines)) as _i:
                _body_once(nc, tc, mybir, masks, *pools,
                           qt_d, kt_d, vv_d, out_d, p_dt, ebias, opts)
        else:
            for _rep in range(reps):
                _body_once(nc, tc, mybir, masks, *pools,
                           qt_d, kt_d, vv_d, out_d, p_dt, ebias, opts)

    nc.compile()
    return nc


def _body_once(nc, tc, mybir, masks, kt_pool, vv_pool, qt_pool, pt_pool,
               osb_pool, st_pool, acc_pool, qt_d, kt_d, vv_d, out_d,
               p_dt, ebias, opts=None):
    opts = opts or {}
    fp32 = mybir.dt.float32
    bf16 = mybir.dt.bfloat16
    for b in range(B):
        ktt = kt_pool.tile([128, S], bf16)
        nc.sync.dma_start(ktt[:], kt_d[:, b * S:(b + 1) * S])
        vvt = vv_pool.tile([128, NT * VW], bf16)
        nc.sync.dma_start(vvt[:], vv_d[b])
        for h in range(HQ):
            qtt = qt_pool.tile([128, S], bf16)
            nc.sync.dma_start(qtt[:], qt_d[h, :, b * S:(b + 1) * S])
            for g in range(NG):
                q0 = 512 * g
                # acc super-tile: 2 PSUM banks; bank0 holds q-tiles
                # 4g,4g+1 at cols 0/129, bank1 holds 4g+2,4g+3 at
                # cols 512/641. First matmul into each bank must carry
                # start=True (whole-bank has_written clear).
                acc = acc_pool.tile([128, 1024], fp32, tag="acc",
                                    name=f"acc_{b}_{h}_{g}")
                bank_cleared = [False, False]
                for (ja, na, qva, qea), (jb, nb, qvb, qeb) in _pairs_for_g(g):
                    colb = na if na + nb <= 512 else 512
                    st = st_pool.tile([128, 1024], fp32)
                    pt = pt_pool.tile([128, 1024], p_dt)
                    nc.tensor.matmul(
                        st[:, 0:na],
                        ktt[:, 128 * ja:128 * ja + 128],
                        qtt[:, qva:qea],
                        start=True, stop=True,
                    )
                    nc.tensor.matmul(
                        st[:, colb:colb + nb],
                        ktt[:, 128 * jb:128 * jb + 128],
                        qtt[:, qvb:qeb],
                        start=True, stop=True,
                    )
                    # exp over the pair; one ACT inst when contiguous
                    if colb == na:
                        nc.scalar.activation(
                            pt[:, 0:na + nb], st[:, 0:na + nb],
                            mybir.ActivationFunctionType.Exp,
                            bias=ebias[:], scale=SCALE)
                    elif na == 512:
                        nc.scalar.activation(
                            pt[:, 0:512 + nb], st[:, 0:512 + nb],
                            mybir.ActivationFunctionType.Exp,
                            bias=ebias[:], scale=SCALE)
                    else:
                        nc.scalar.activation(
                            pt[:, 0:na], st[:, 0:na],
                            mybir.ActivationFunctionType.Exp,
                            bias=ebias[:], scale=SCALE)
                        nc.scalar.activation(
                            pt[:, 512:512 + nb], st[:, 512:512 + nb],
                            mybir.ActivationFunctionType.Exp,
                            bias=ebias[:], scale=SCALE)
                    for j, n, qv, qe, col in ((ja, na, qva, qea, 0),
                                              (jb, nb, qvb, qeb, colb)):
                        if j >= 4 * g:
                            # causal diagonal tile: first 128 cols
                            nc.vector.tensor_mul(
                                pt[:, col:col + 128], pt[:, col:col + 128],
                                masks[:, 0:128])
                        if qe == 128 * j + 128 + W:
                            # window edge tile: last 128 cols
                            nc.vector.tensor_mul(
                                pt[:, col + n - 128:col + n],
                                pt[:, col + n - 128:col + n],
                                masks[:, 128:256])
                        for i in range(max(4 * g, j), min(4 * g + 3, j + 8) + 1):
                            s_ = i - 4 * g
                            off = col + 128 * i - qv
                            a0 = 512 * (s_ // 2) + VW * (s_ % 2)
                            bank = s_ // 2
                            nc.tensor.matmul(
                                acc[:, a0:a0 + VW],
                                pt[:, off:off + 128],
                                vvt[:, VW * j:VW * j + VW],
                                start=not bank_cleared[bank],
                                stop=(j == i),
                                skip_group_check=True,
                            )
                            bank_cleared[bank] = True
                            if j == i and s_ % 2 == 1:
                                # both q-tiles of this bank are done:
                                # raw copy (num|den pairs) and ship out
                                half = s_ // 2
                                ot = osb_pool.tile([128, OW], fp32)
                                nc.vector.tensor_copy(
                                    ot[:], acc[:, 512 * half:512 * half + OW])
                                nc.sync.dma_start(
                                    out_d[h, b, g, half], ot[:])


def _mask_np(dtype):
    """[128, 256]: cols 0:128 diag keep r<=c; cols 128:256 edge keep c<r."""
    r = np.arange(128)[:, None]
    c = np.arange(128)[None, :]
    diag = (r <= c).astype(np.float32)
    edge = (c < r).astype(np.float32)
    return np.concatenate([diag, edge], axis=1).astype(dtype)


def _prep_in_maps(query, key, value, fp8_p=False):
    q = np.asarray(query, dtype=np.float32).reshape(B, S, H, D)
    k = np.asarray(key, dtype=np.float32).reshape(B, S, KVH, D)
    v = np.asarray(value, dtype=np.float32).reshape(B, S, KVH, D)

    # [H, D, B*S] / [KVH, D, B*S]
    qt_all = np.ascontiguousarray(q.transpose(2, 3, 0, 1).reshape(H, D, BS)).astype(_BF16)
    kt_all = np.ascontiguousarray(k.transpose(2, 3, 0, 1).reshape(KVH, D, BS)).astype(_BF16)

    # V with ones column, packed [KVH, B, 128p, NT*VW] so that
    # vv[c, b, p, t*VW + d] = V'[b, 128t + p, c, d]
    vpad = np.concatenate([v, np.ones((B, S, KVH, 1), np.float32)], axis=3)
    vv_all = np.ascontiguousarray(
        vpad.reshape(B, NT, 128, KVH, VW).transpose(3, 0, 2, 1, 4)
        .reshape(KVH, B, 128, NT * VW)).astype(_BF16)

    mk = _mask_np(_FP8 if fp8_p else _BF16)
    return [
        {
            "qt": np.ascontiguousarray(qt_all[HQ * c:HQ * c + HQ]),
            "kt": np.ascontiguousarray(kt_all[c]),
            "vv": np.ascontiguousarray(vv_all[c]),
            "mk": mk,
        }
        for c in range(N_CORES)
    ]


def _assemble(results):
    # results[c]["out"]: [HQ, B, NG, 2, 128, 258] raw (num|den) pairs.
    o = np.stack([np.asarray(results[c]["out"], dtype=np.float32)
                  for c in range(N_CORES)])  # [8, HQ, B, NG, 2, 128, 258]
    o = o.reshape(N_CORES, HQ, B, NG, 2, 128, 2, VW)
    num = o[..., 0:D]            # [8, HQ, B, NG, 2, 128, 2, 128]
    den = o[..., D:D + 1]
    out = num / den              # normalized
    # index order: [c, h, b, g, half, p, sub, d] with q = 512g + 256*half
    # ... wait: q-tile i = 4g + 2*half + sub, q = 128*i + p
    out = out.transpose(2, 3, 4, 6, 5, 0, 1, 7)  # [B, NG, 2, 2, 128, 8, HQ, D]
    return np.ascontiguousarray(out.reshape(B, S, H * D))


def kernel(query, key, value):
    from concourse import bass_utils

    if "nc" not in _CACHE:
        _CACHE["nc"] = _build_nc()
    nc = _CACHE["nc"]
    in_maps = _prep_in_maps(query, key, value)
    res = bass_utils.run_bass_kernel_spmd(
        nc, in_maps, core_ids=list(range(N_CORES)))
    return _assemble(res.results)


# revision 10
# speedup vs baseline: 1.1943x; 1.1943x over previous
"""Trainium2 Bass kernel for GQA sliding-window causal attention.

Problem: B=2, S=2048, H=32 q-heads, KVH=8 kv-heads, D=128,
sliding window 1024, causal, scale 1/sqrt(128). f32 I/O.

Sharding (8 cores, pure tensor parallel, no collectives): core c gets
kv-head c and its query-head group [4c, 4c+4). Each core computes full
attention for its 4 q-heads over both batch elements; host concatenates
along the head dim.

Per-core algorithm (banded, no online softmax needed since scores are
O(1) and exp never overflows):
  - Q and K live in SBUF transposed: [d=128 partitions, s free].
  - Scores computed transposed, ST[k, q] = (KT_j).T-contracted-with-QT,
    per (512-wide q-block, 128-wide k-tile) over the causal+window band.
    Two consecutive k-tiles share one 2-bank PSUM mega-tile so a single
    ScalarE activation handles both (amortizes the ~300-cycle ACT
    instruction overhead).
  - P = exp(SCALE * ST - 2) on ScalarE, written as fp8e4 to SBUF. The
    -2 offset keeps exp below the fp8e4 max (240); it cancels in the
    final normalization.
  - Causal-diagonal and window-edge tiles are masked AFTER exp by
    multiplying with 0/1 fp8 mask tiles on VectorE (exact zeros).
  - PV: acc[q, 0:129] += PT_slice.T @ V'_j where PT is fp8 (stationary,
    4x faster weight load) and V' is bf16 with a ones column appended
    -> col 128 accumulates the softmax denominator for free.
    Two q-tiles' accumulators share one PSUM bank (2*129 <= 512); the
    chronologically first matmul into a bank carries start=True (whole-
    bank has_written clear), everything after accumulates per element.
  - acc banks are copied raw (numerator + denominator) to SBUF on DVE
    and DMA'd out; the division happens on the host during unsharding.
All matmuls accumulate f32 in PSUM; softmax math in f32 on ScalarE.
"""

import numpy as np
import ml_dtypes

B = 2
S = 2048
H = 32
KVH = 8
D = 128
HQ = H // KVH  # q heads per core = 4
W = 1024  # sliding window
SCALE = 0.08838834764831845
EXP_BIAS = -3.5  # folded into exp; cancels in normalization.
# Keeps exp below the fp8e4 max (240): observed max scaled score is ~7.8
# (driven by aligned large-norm q/k rows), overflow would need > 8.98.
N_CORES = 8
BS = B * S  # 4096
NT = S // 128  # 16 k-tiles / q-tiles per sequence
NG = S // 512  # 4 q-blocks per sequence
VW = D + 1  # 129: V width with ones column
OW = 2 * VW  # 258: two packed (num|den) column groups per PSUM bank

_BF16 = ml_dtypes.bfloat16
_FP8 = ml_dtypes.float8_e4m3

_CACHE = {}


def _pairs_for_g(g):
    """Consecutive-j pairs for q-block g, larger-n first within a pair.

    Returns list of [(j, n, qv, qe), (j, n, qv, qe)] pairs covering the
    causal+window band for q-range [512g, 512g+512).
    """
    q0 = 512 * g
    tiles = []
    for j in range(max(0, 4 * g - 8), 4 * g + 4):
        qv = max(q0, 128 * j)
        qe = min(q0 + 512, 128 * j + 128 + W)
        tiles.append((j, qe - qv, qv, qe))
    pairs = []
    for t in range(0, len(tiles), 2):
        a, b = tiles[t], tiles[t + 1]
        if a[1] < b[1]:
            a, b = b, a  # larger n first: avoids bank-crossing placement
        pairs.append((a, b))
    return pairs


def _build_nc(reps=1, loop_reps=0, opts=None):
    """Build + compile the single-core Bass/Tile program (SPMD across 8).

    reps > 1 unrolls the whole computation inside one NEFF; loop_reps > 0
    instead wraps the body in a hardware For_i loop. Both are used only
    for timing. opts: dict of tuning switches (see _body_once).
    """
    from contextlib import ExitStack

    import concourse.bass as bass
    import concourse.tile as tile
    from concourse import bacc, mybir

    opts = dict(opts or {})
    fp32 = mybir.dt.float32
    bf16 = mybir.dt.bfloat16
    fp8 = mybir.dt.float8e4
    p_dt = fp8 if opts.get("fp8_p", False) else bf16

    nc = bacc.Bacc("TRN2", target_bir_lowering=False, debug=False,
                   num_devices=N_CORES)

    qt_d = nc.dram_tensor("qt", [HQ, D, BS], bf16, kind="ExternalInput").ap()
    kt_d = nc.dram_tensor("kt", [D, BS], bf16, kind="ExternalInput").ap()
    vv_d = nc.dram_tensor("vv", [B, 128, NT * VW], bf16, kind="ExternalInput").ap()
    mk_d = nc.dram_tensor("mk", [128, 256], p_dt, kind="ExternalInput").ap()
    out_d = nc.dram_tensor("out", [HQ, B, NG, 2, 128, OW], fp32,
                           kind="ExternalOutput").ap()

    with tile.TileContext(nc) as tc, ExitStack() as ctx:
        mask_pool = ctx.enter_context(tc.tile_pool(name="mask", bufs=1))
        kt_pool = ctx.enter_context(tc.tile_pool(name="ktp", bufs=2))
        vv_pool = ctx.enter_context(tc.tile_pool(name="vvp", bufs=2))
        qt_pool = ctx.enter_context(tc.tile_pool(name="qtp", bufs=2))
        pt_pool = ctx.enter_context(
            tc.tile_pool(name="ptp", bufs=opts.get("pt_bufs", 6)))
        osb_pool = ctx.enter_context(tc.tile_pool(name="osb", bufs=4))
        st_pool = ctx.enter_context(
            tc.tile_pool(name="stp", bufs=opts.get("st_bufs", 2), space="PSUM"))
        acc_pool = ctx.enter_context(
            tc.tile_pool(name="accp", bufs=opts.get("acc_bufs", 2),
                         space="PSUM"))

        masks = mask_pool.tile([128, 256], p_dt)
        nc.sync.dma_start(masks[:], mk_d[:])
        ebias = mask_pool.tile([128, 1], fp32, name="ebias")
        nc.vector.memset(ebias[:], EXP_BIAS)

        pools = (kt_pool, vv_pool, qt_pool, pt_pool, osb_pool, st_pool,
                 acc_pool)
        if loop_reps:
            with tc.For_i(0, loop_reps, 1,
                          hint_engines=tuple(nc.engines)) as _i:
                _body_once(nc, tc, mybir, masks, *pools,
                           qt_d, kt_d, vv_d, out_d, p_dt, ebias, opts)
        else:
            for _rep in range(reps):
                _body_once(nc, tc, mybir, masks, *pools,
                           qt_d, kt_d, vv_d, out_d, p_dt, ebias, opts)

    nc.compile()
    return nc


def _body_once(nc, tc, mybir, masks, kt_pool, vv_pool, qt_pool, pt_pool,
               osb_pool, st_pool, acc_pool, qt_d, kt_d, vv_d, out_d,
               p_dt, ebias, opts=None):
    opts = opts or {}
    fp32 = mybir.dt.float32
    bf16 = mybir.dt.bfloat16
    for b in range(B):
        ktt = kt_pool.tile([128, S], bf16)
        nc.sync.dma_start(ktt[:], kt_d[:, b * S:(b + 1) * S])
        vvt = vv_pool.tile([128, NT * VW], bf16)
        nc.sync.dma_start(vvt[:], vv_d[b])
        for h in range(HQ):
            qtt = qt_pool.tile([128, S], bf16)
            nc.sync.dma_start(qtt[:], qt_d[h, :, b * S:(b + 1) * S])
            for g in range(NG):
                q0 = 512 * g
                # acc super-tile: 2 PSUM banks; bank0 holds q-tiles
                # 4g,4g+1 at cols 0/129, bank1 holds 4g+2,4g+3 at
                # cols 512/641. First matmul into each bank must carry
                # start=True (whole-bank has_written clear).
                acc = acc_pool.tile([128, 1024], fp32, tag="acc",
                                    name=f"acc_{b}_{h}_{g}")
                bank_cleared = [False, False]
                for (ja, na, qva, qea), (jb, nb, qvb, qeb) in _pairs_for_g(g):
                    colb = na if na + nb <= 512 else 512
                    st = st_pool.tile([128, 1024], fp32)
                    pt = pt_pool.tile([128, 1024], p_dt)
                    nc.tensor.matmul(
                        st[:, 0:na],
                        ktt[:, 128 * ja:128 * ja + 128],
                        qtt[:, qva:qea],
                        start=True, stop=True,
                    )
                    nc.tensor.matmul(
                        st[:, colb:colb + nb],
                        ktt[:, 128 * jb:128 * jb + 128],
                        qtt[:, qvb:qeb],
                        start=True, stop=True,
                    )
                    # exp over the pair; one ACT inst when contiguous
                    if colb == na:
                        nc.scalar.activation(
                            pt[:, 0:na + nb], st[:, 0:na + nb],
                            mybir.ActivationFunctionType.Exp,
                            bias=ebias[:], scale=SCALE)
                    elif na == 512:
                        nc.scalar.activation(
                            pt[:, 0:512 + nb], st[:, 0:512 + nb],
                            mybir.ActivationFunctionType.Exp,
                            bias=ebias[:], scale=SCALE)
                    else:
                        nc.scalar.activation(
                            pt[:, 0:na], st[:, 0:na],
                            mybir.ActivationFunctionType.Exp,
                            bias=ebias[:], scale=SCALE)
                        nc.scalar.activation(
                            pt[:, 512:512 + nb], st[:, 512:512 + nb],
                            mybir.ActivationFunctionType.Exp,
                            bias=ebias[:], scale=SCALE)
                    for j, n, qv, qe, col in ((ja, na, qva, qea, 0),
                                              (jb, nb, qvb, qeb, colb)):
                        if j >= 4 * g:
                            # causal diagonal tile: first 128 cols
                            nc.vector.tensor_mul(
                                pt[:, col:col + 128], pt[:, col:col + 128],
                                masks[:, 0:128])
                        if qe == 128 * j + 128 + W:
                            # window edge tile: last 128 cols
                            nc.vector.tensor_mul(
                                pt[:, col + n - 128:col + n],
                                pt[:, col + n - 128:col + n],
                                masks[:, 128:256])
                        for i in range(max(4 * g, j), min(4 * g + 3, j + 8) + 1):
                            s_ = i - 4 * g
                            off = col + 128 * i - qv
                            a0 = 512 * (s_ // 2) + VW * (s_ % 2)
                            bank = s_ // 2
                            nc.tensor.matmul(
                                acc[:, a0:a0 + VW],
                                pt[:, off:off + 128],
                                vvt[:, VW * j:VW * j + VW],
                                start=not bank_cleared[bank],
                                stop=(j == i),
                                skip_group_check=True,
                            )
                            bank_cleared[bank] = True
                            if j == i and s_ % 2 == 1:
                                # both q-tiles of this bank are done:
                                # raw copy (num|den pairs) and ship out
                                half = s_ // 2
                                ot = osb_pool.tile([128, OW], fp32)
                                nc.vector.tensor_copy(
                                    ot[:], acc[:, 512 * half:512 * half + OW])
                                nc.sync.dma_start(
                                    out_d[h, b, g, half], ot[:])


def _mask_np(dtype):
    """[128, 256]: cols 0:128 diag keep r<=c; cols 128:256 edge keep c<r."""
    r = np.arange(128)[:, None]
    c = np.arange(128)[None, :]
    diag = (r <= c).astype(np.float32)
    edge = (c < r).astype(np.float32)
    return np.concatenate([diag, edge], axis=1).astype(dtype)


def _prep_in_maps(query, key, value, fp8_p=False):
    q = np.asarray(query, dtype=np.float32).reshape(B, S, H, D)
    k = np.asarray(key, dtype=np.float32).reshape(B, S, KVH, D)
    v = np.asarray(value, dtype=np.float32).reshape(B, S, KVH, D)

    # [H, D, B*S] / [KVH, D, B*S]
    qt_all = np.ascontiguousarray(q.transpose(2, 3, 0, 1).reshape(H, D, BS)).astype(_BF16)
    kt_all = np.ascontiguousarray(k.transpose(2, 3, 0, 1).reshape(KVH, D, BS)).astype(_BF16)

    # V with ones column, packed [KVH, B, 128p, NT*VW] so that
    # vv[c, b, p, t*VW + d] = V'[b, 128t + p, c, d]
    vpad = np.concatenate([v, np.ones((B, S, KVH, 1), np.float32)], axis=3)
    vv_all = np.ascontiguousarray(
        vpad.reshape(B, NT, 128, KVH, VW).transpose(3, 0, 2, 1, 4)
        .reshape(KVH, B, 128, NT * VW)).astype(_BF16)

    mk = _mask_np(_FP8 if fp8_p else _BF16)
    return [
        {
            "qt": np.ascontiguousarray(qt_all[HQ * c:HQ * c + HQ]),
            "kt": np.ascontiguousarray(kt_all[c]),
            "vv": np.ascontiguousarray(vv_all[c]),
            "mk": mk,
        }
        for c in range(N_CORES)
    ]


def _assemble(results):
    # results[c]["out"]: [HQ, B, NG, 2, 128, 258] raw (num|den) pairs.
    o = np.stack([np.asarray(results[c]["out"], dtype=np.float32)
                  for c in range(N_CORES)])  # [8, HQ, B, NG, 2, 128, 258]
    o = o.reshape(N_CORES, HQ, B, NG, 2, 128, 2, VW)
    num = o[..., 0:D]            # [8, HQ, B, NG, 2, 128, 2, 128]
    den = o[..., D:D + 1]
    out = num / den              # normalized
    # index order: [c, h, b, g, half, p, sub, d] with q = 512g + 256*half
    # ... wait: q-tile i = 4g + 2*half + sub, q = 128*i + p
    out = out.transpose(2, 3, 4, 6, 5, 0, 1, 7)  # [B, NG, 2, 2, 128, 8, HQ, D]
    return np.ascontiguousarray(out.reshape(B, S, H * D))


def kernel(query, key, value):
    from concourse import bass_utils

    if "nc" not in _CACHE:
        _CACHE["nc"] = _build_nc()
    nc = _CACHE["nc"]
    in_maps = _prep_in_maps(query, key, value)
    res = bass_utils.run_bass_kernel_spmd(
        nc, in_maps, core_ids=list(range(N_CORES)))
    return _assemble(res.results)
